# revision 1
# baseline (speedup 1.0000x reference)
"""Trainium2 Bass kernel for nn_CausalAttention (no actual causal mask, per the
reference bug): out = softmax((x@Wq)(x@Wk)^T / 64**0.05) @ (x@Wv).

Sharding: data-parallel over batch, one batch element per NeuronCore (B=8, 8 cores).
Per core, a flash-attention-style loop over k-chunks with *transposed* scores
(sT[k, q]) so the probability tiles come out of the exp in exactly the layout the
P@V matmul needs as its stationary operand (no per-tile transposes of P).

Numerics / dtype choices (all matmuls accumulate in fp32 PSUM):
 - x is shipped from the host as fp16 (2^-11 relative representation error) in
   feature-chunk-major layout so the xbar DMA transpose (2-byte only) can load
   x^T directly.  fp16 operands stream through the PE at 1 col/cycle.
 - probabilities P = exp(s/SCALE - 25) are written as bf16 (fp16 lacks the
   range).  Rounding P is benign: the ones-column of v_aug makes the softmax
   denominator the sum of the *same* rounded weights, so out stays a proper
   weighted average of v.
 - softmax skips the max-subtraction pass: scores/SCALE are bounded well inside
   fp32 exp range for randn inputs, and the -25 shift gives extra headroom.

Perf details encoded here:
 - q^T/k^T are produced *duplicated* across both partition halves (the
   projection uses doubled weights, M=128) so the K=64 QK^T matmuls can be
   row-paired with tile_position: two k-chunks run concurrently in the two
   row-halves of the PE array.
 - the PE HAM clock-gate does not treat half-array matmuls (K=64 or M=65) as
   activity, so phase 2 would run at 1.2 GHz; a tiny full-array "heater"
   matmul per exp-window keeps the PE at 2.4 GHz.
 - all DMA-transposes issue from ONE HWDGE engine; concurrent transposes from
   the sync and scalar rings corrupt data in the shared XBAR (verified).
"""

import sys

import numpy as np

for _p in ("/root/.axon_site", "/root/.axon_site/_ro/trn_rl_repo",
           "/root/.axon_site/_ro/pypackages", "/opt/trn_rl_repo"):
    if _p not in sys.path:
        sys.path.append(_p)

B, S, D, H = 8, 4096, 768, 64
P = 128
SCALE = float(H) ** 0.05
EXP_SHIFT = -25.0

_cached = {}


def build_program(S=S, D=D, H=H, SC=1024, QC=512, WIN=3, pair_qk=True,
                  heater=True):
    import concourse.mybir as mybir
    import concourse.tile as tile
    from concourse import bacc
    from concourse.masks import make_identity

    NF = D // P          # feature chunks
    NSC = S // SC        # phase-1 s-chunks
    KC = S // P          # k-chunks
    NQC = S // QC        # phase-2 q-chunks

    f32 = mybir.dt.float32
    f16 = mybir.dt.float16
    bf16 = mybir.dt.bfloat16

    nc = bacc.Bacc("TRN2", target_bir_lowering=False)

    x_d = nc.dram_tensor("x16", [NF, S, P], f16, kind="ExternalInput")
    wq_d = nc.dram_tensor("wq", [D, H], f32, kind="ExternalInput")
    wk_d = nc.dram_tensor("wk", [D, H], f32, kind="ExternalInput")
    wv_d = nc.dram_tensor("wv", [D, H], f32, kind="ExternalInput")
    out_d = nc.dram_tensor("out", [S, H], f32, kind="ExternalOutput")

    QP = P if pair_qk else H  # partition extent of qT/kT (duplicated if paired)

    with tile.TileContext(nc) as tc:
        with (
            tc.tile_pool(name="persist", bufs=1) as persist,
        ):
            qT = persist.tile([QP, S], f16)         # q^T, d on partitions
            kT = persist.tile([QP, S], f16)
            v_aug = persist.tile([P, KC, H + 1], f16)  # [k-part, chunk, v | ones]
            w_stage = persist.tile([P, 3, NF, H], f32)
            # wq/wk chunks duplicated along M so the projection directly
            # writes q^T/k^T into both partition halves
            w_sb = persist.tile([P, 2, NF, QP], f16)
            wv_sb = persist.tile([P, NF, H], f16)
            ident = persist.tile([P, P], f32)
            exp_bias = persist.tile([P, 1], f32)
            heat = persist.tile([P, P], f16)

            make_identity(nc, ident)
            nc.vector.memset(v_aug[:, :, H:H + 1], 1.0)
            nc.vector.memset(exp_bias, EXP_SHIFT)
            nc.vector.memset(heat, 0.001)
            for i, w_d in enumerate((wq_d, wk_d, wv_d)):
                nc.sync.dma_start(
                    w_stage[:, i], w_d[:].rearrange("(g p) h -> p g h", p=P)
                )
            for i in range(2):
                nc.vector.tensor_copy(w_sb[:, i, :, 0:H], w_stage[:, i])
                if pair_qk:
                    nc.vector.tensor_copy(w_sb[:, i, :, H:2 * H], w_stage[:, i])
            nc.vector.tensor_copy(wv_sb[:], w_stage[:, 2])

            # ---------------- Phase 1: x^T + projections ----------------
            with (
                tc.tile_pool(name="xts", bufs=3) as xts,
                tc.tile_pool(name="p1psum", bufs=2, space="PSUM") as p1psum,
                tc.tile_pool(name="p1psv", bufs=2, space="PSUM") as p1psv,
            ):
                for c in range(NSC):
                    with nc.named_scope(f"p1_c{c}"):
                        sl = slice(c * SC, (c + 1) * SC)
                        xf = xts.tile([P, NF, SC], f16, tag="xf")
                        for g in range(NF):
                            nc.sync.dma_start_transpose(xf[:, g], x_d[g, sl, :])
                        # q^T and k^T chunks (duplicated into both halves)
                        for wi, dest in ((0, qT), (1, kT)):
                            for half in range(SC // 512):
                                hs = slice(half * 512, (half + 1) * 512)
                                ps = p1psum.tile([QP, 512], f32, tag="proj")
                                for g in range(NF):
                                    nc.tensor.matmul(
                                        ps, w_sb[:, wi, g], xf[:, g, hs],
                                        start=(g == 0), stop=(g == NF - 1),
                                    )
                                nc.vector.tensor_copy(
                                    dest[:, c * SC + half * 512:
                                         c * SC + (half + 1) * 512], ps
                                )
                        # v chunks: [128, 64] = x @ Wv
                        for t in range(SC // P):
                            ps = p1psv.tile([P, H], f32, tag="vproj")
                            for g in range(NF):
                                nc.tensor.matmul(
                                    ps, xf[:, g, t * P:(t + 1) * P],
                                    wv_sb[:, g],
                                    start=(g == 0), stop=(g == NF - 1),
                                )
                            nc.vector.tensor_copy(
                                v_aug[:, c * (SC // P) + t, 0:H], ps
                            )

            # ---------------- Phase 2: attention ----------------
            with (
                tc.tile_pool(name="pt", bufs=4) as ptp,
                tc.tile_pool(name="drain", bufs=2) as drainp,
                tc.tile_pool(name="stpsum", bufs=2, space="PSUM") as stpsum,
                tc.tile_pool(name="opsum", bufs=2, space="PSUM") as opsum,
            ):
                # flat window list across all q-chunks: (qc, k, w)
                windows = []
                for qc in range(NQC):
                    k = 0
                    while k < KC:
                        w = min(WIN, KC - k)
                        windows.append((qc, k, w))
                        k += w

                o_tiles = {}

                def emit_qk(qc, k, w):
                    st = stpsum.tile([P, WIN, QC], f32, tag="st")
                    if heater:
                        # full-array dummy matmul: keeps the PE HAM clock-gate
                        # at 2.4 GHz (half-array matmuls don't register as
                        # activity).
                        nc.tensor.matmul(
                            st[:, 0, 0:P], heat, heat, start=True, stop=True,
                        )
                    for j in range(w):
                        kj = k + j
                        if pair_qk:
                            hp = (kj % 2) * H  # partition half
                            nc.tensor.matmul(
                                st[:, j],
                                kT[hp:hp + H, kj * P:(kj + 1) * P],
                                qT[hp:hp + H, qc * QC:(qc + 1) * QC],
                                start=True, stop=True,
                                tile_position=(hp, 0),
                            )
                        else:
                            nc.tensor.matmul(
                                st[:, j],
                                kT[:, kj * P:(kj + 1) * P],
                                qT[:, qc * QC:(qc + 1) * QC],
                                start=True, stop=True,
                            )
                    return st

                def emit_exp(st, w):
                    pt = ptp.tile([P, WIN, QC], bf16, tag="pt")
                    nc.scalar.activation(
                        pt[:, :w], st[:, :w],
                        mybir.ActivationFunctionType.Exp,
                        bias=exp_bias, scale=1.0 / SCALE,
                    )
                    return pt

                def emit_pv(qc, k, w, pt):
                    if k == 0:
                        o_tiles[qc] = opsum.tile([H + 1, QC], f32, tag="o", name="o_ps")
                    for j in range(w):
                        nc.tensor.matmul(
                            o_tiles[qc], v_aug[:, k + j], pt[:, j],
                            start=(k + j == 0), stop=(k + j == KC - 1),
                            skip_group_check=True,
                        )

                def emit_drain(qc):
                    # outT [65, QC] -> transpose 128-blocks -> normalize -> DMA
                    o_ps = o_tiles.pop(qc)
                    oT = drainp.tile([H + 1, QC], f32, tag="oT")
                    nc.vector.tensor_copy(oT, o_ps)
                    t_ps = opsum.tile([P, QC // P, H + 1], f32, tag="o")
                    if heater:
                        nc.tensor.matmul(
                            t_ps[:, 0, :], heat[:, 0:P], heat[:, 0:H + 1],
                            start=True, stop=True,
                        )
                    stage = drainp.tile([P, QC // P, H], f32, tag="stage")
                    for j in range(QC // P):
                        nc.tensor.transpose(
                            t_ps[:, j], oT[:, j * P:(j + 1) * P],
                            ident[:H + 1, :H + 1],
                        )
                        rz = drainp.tile([P, 1], f32, tag="rz")
                        nc.vector.reciprocal(rz, t_ps[:, j, H:H + 1])
                        nc.vector.tensor_scalar_mul(
                            stage[:, j], t_ps[:, j, 0:H], rz
                        )
                    nc.sync.dma_start(
                        out_d[qc * QC:(qc + 1) * QC, :].rearrange(
                            "(j p) h -> p j h", p=P
                        ),
                        stage,
                    )

                # Global software pipeline: PE always has QK(w+1) queued ahead
                # of PV(w), even across q-chunk boundaries, so the strict-FIFO
                # PE queue never waits on the exp of the current window; each
                # chunk's drain is emitted after its last PV.
                prev = None
                st_tiles = {}
                for i, (qc, k, w) in enumerate(windows):
                    with nc.named_scope(f"p2_q{qc}_k{k}"):
                        if i not in st_tiles:
                            st_tiles[i] = emit_qk(qc, k, w)
                        if k + w == KC and i + 1 < len(windows):
                            # boundary: pre-issue the next chunk's first QK so
                            # its exp isn't delayed behind this chunk's last PV
                            nq, nk, nw = windows[i + 1]
                            st_tiles[i + 1] = emit_qk(nq, nk, nw)
                        if prev is not None:
                            emit_pv(prev[0], prev[1], prev[2], prev[3])
                            if prev[1] + prev[2] == KC:
                                emit_drain(prev[0])
                        pt = emit_exp(st_tiles.pop(i), w)
                        prev = (qc, k, w, pt)
                with nc.named_scope("p2_tail"):
                    emit_pv(prev[0], prev[1], prev[2], prev[3])
                    emit_drain(prev[0])

    nc.compile()
    return nc


def make_host_inputs(x):
    """fp16 cast of x, rearranged feature-chunk-major so each [S, 128] slab is
    contiguous for the xbar DMA transpose. x: [..., S, D]."""
    s, d = x.shape[-2], x.shape[-1]
    lead = x.shape[:-2]
    nf = d // P
    x16 = x.astype(np.float16).reshape(*lead, s, nf, P).swapaxes(-2, -3)
    return np.ascontiguousarray(x16)


def kernel(x, W_q, W_k, W_v):
    from concourse.bass_utils import run_bass_kernel_spmd

    x = np.ascontiguousarray(np.asarray(x, dtype=np.float32))
    W_q = np.ascontiguousarray(np.asarray(W_q, dtype=np.float32))
    W_k = np.ascontiguousarray(np.asarray(W_k, dtype=np.float32))
    W_v = np.ascontiguousarray(np.asarray(W_v, dtype=np.float32))

    x16 = make_host_inputs(x)

    if "nc" not in _cached:
        _cached["nc"] = build_program()
    nc = _cached["nc"]

    in_maps = [
        {
            "x16": x16[c],
            "wq": W_q,
            "wk": W_k,
            "wv": W_v,
        }
        for c in range(B)
    ]
    res = run_bass_kernel_spmd(nc, in_maps, core_ids=list(range(B)))
    _cached["last_res"] = res
    return np.stack([r["out"] for r in res.results], axis=0)


if __name__ == "__main__":
    rng = np.random.default_rng(0)
    x = rng.standard_normal((B, S, D), dtype=np.float32)
    Wq = rng.standard_normal((D, H), dtype=np.float32) * D ** -0.5
    Wk = rng.standard_normal((D, H), dtype=np.float32) * D ** -0.5
    Wv = rng.standard_normal((D, H), dtype=np.float32) * D ** -0.5
    out = kernel(x, Wq, Wk, Wv)
    print(out.shape, out.dtype)



# revision 3
# speedup vs baseline: 1.0300x; 1.0300x over previous
"""Trainium2 Bass kernel for nn_CausalAttention (no actual causal mask, per the
reference bug): out = softmax((x@Wq)(x@Wk)^T / 64**0.05) @ (x@Wv).

Sharding: data-parallel over batch, one batch element per NeuronCore (B=8).

Key structure (v2 — dual-engine softmax + phase overlap):
 - Scores are produced in "bf16-bit units": W_q is pre-scaled on the HOST by
   128*log2(e)/SCALE, so the QK^T PSUM value s_b satisfies
   exp(s/SCALE) = 2^(s_b/128).  This lets the exp be computed two ways:
     * ACT (scalar) engine: exact spline exp with scale=ln2/128, bias=-25.
     * DVE (vector) engine: a CUSTOM 8-stage ALU op (EXP2_BITS_ANT) that
       computes the bf16 BIT PATTERN of 2^(s_b/128 - const) directly:
       magic-number floor-split + quadratic mantissa-hump correction,
       written as uint16 and bit-viewed as bf16 (max rel err ~0.6%, on par
       with the bf16 quantization the exact path pays anyway).
   Windows alternate between the two engines, nearly doubling softmax
   throughput — the scalar engine was the phase-2 bottleneck (100% busy).
 - Phase 1 (x^T DMA transposes + q/k/v projections) is interleaved into the
   first quarter of phase 2 so the serialized xbar transposes (~8.1us per
   1024-row chunk on the one safe DMA ring) hide behind attention windows.
   Attention runs on q-chunk PAIRS; the first pair's windows are gated on
   k-availability as x^T chunks land.
 - q^T/k^T are produced in ONE projection pass with stationary [Wq|Wk]
   (M=128, no duplicated weights); the "swapped-half" copy needed for
   row-paired QK^T matmuls (two K=64 matmuls in the two PE row halves via
   tile_position) is made by two cheap DVE half-copies.
 - v_aug is padded to M=128 ([v | ones | zeros]) so the PV matmuls are
   full-array (K=128, M=128) and keep the PE HAM clock-gate at 2.4 GHz
   without dedicated heater matmuls.
 - probabilities bf16, x fp16 (host-preformatted feature-chunk-major for the
   2-byte xbar DMA transpose), all matmuls accumulate in fp32 PSUM; softmax
   denominator comes free via the ones column (sum of the same rounded
   weights -> output stays a proper weighted average).
"""

import sys

import numpy as np

for _p in ("/root/.axon_site", "/root/.axon_site/_ro/trn_rl_repo",
           "/root/.axon_site/_ro/pypackages", "/opt/trn_rl_repo"):
    if _p not in sys.path:
        sys.path.append(_p)

B, S, D, H = 8, 4096, 768, 64
P = 128
SCALE = float(H) ** 0.05
LOG2E = 1.4426950408889634
QBIT = 128.0 * LOG2E / SCALE      # host pre-scale folded into W_q
EXP_SHIFT = -25.0                 # common shift, cancels in softmax

# EXP2_BITS_ANT constants (see probe: fp32->uint16 cast rounds to nearest)
E2_C2 = 0.00265                                      # mantissa-hump quad coef
E2_C0 = 128.0 * (127.0 + EXP_SHIFT * LOG2E) - 64.0   # bias - 64 (floor split)
E2_C1 = 1.5 * 2.0 ** 30                              # magic (rounds to k*128)
E2_C3 = (64.0 - 4096.0 * E2_C2) / E2_C2              # alignment const / coef

_cached = {}


def _register_exp2():
    """Register the custom DVE op computing bf16 bits of 2^((x+C0)/128)+hump.

    bits = Y2 + ((F*F) + C3)*C2 ; Y2 = x + C0 ; F = Y2 - round128(Y2)
    Exactly 8 ALU stages; C3 spilled via in1 per the custom-DVE API.
    """
    from concourse import dve_ops as dvo
    from concourse.dve_spec import (
        C0, C1, C2, C3, Spec, Src0, _spill_c3_to_src1, lower,
    )
    from concourse.dve_uop import DveOpSpec

    for op in dvo.OPS:
        if op.name == "EXP2_BITS_ANT":
            return op

    Y2 = Src0 + C0
    T = Y2 + C1
    N = T - C1
    F = Y2 - N
    Q = F * F
    bits = Y2 + (Q + C3) * C2

    def ref(in0, in1, c0, c1, c2):
        y2 = in0.astype(np.float32) + np.float32(c0)
        t = (y2 + np.float32(c1)).astype(np.float32)
        n = (t - np.float32(c1)).astype(np.float32)
        f = y2 - n
        c3 = np.asarray(in1, np.float32).reshape(in0.shape[0], 1)
        return ((f * f) + c3) * np.float32(c2) + y2

    spec = Spec(body=_spill_c3_to_src1(bits), reference=ref)
    shas = {}
    for ver in ("v3", "v4"):
        uops = lower(spec, ver=ver)
        shas[ver] = DveOpSpec(
            name="EXP2_BITS_ANT", opcode=None, uops=uops, rd1_en=True
        ).sha(ver)
    op = dvo.DveOp("EXP2_BITS_ANT", spec, subdim=False, uops_sha=shas)
    dvo.OPS.append(op)
    dvo.CUSTOM_DVE_SPECS[op.name] = op.spec
    dvo._SUB_OPCODE_FOR_NAME[op.name] = dvo._CUSTOM_DVE_ROW_BASE + len(dvo.OPS) - 1
    return op


def build_program(S=S, D=D, H=H, SC=1024, QC=512, WIN=2, dve_num=1, dve_den=2,
                  heater=False):
    import concourse.mybir as mybir
    import concourse.tile as tile
    from concourse import bacc
    from concourse.masks import make_identity

    EXP2 = _register_exp2()

    NF = D // P          # feature chunks (6)
    NSC = S // SC        # phase-1 s-chunks (4)
    KC = S // P          # k-chunks (32)
    NQC = S // QC        # q-chunks (8)
    KPC = SC // P        # k-chunks per s-chunk (8)
    VA = P               # padded v_aug width (full-array PV)

    f32 = mybir.dt.float32
    f16 = mybir.dt.float16
    bf16 = mybir.dt.bfloat16
    u16 = mybir.dt.uint16

    nc = bacc.Bacc("TRN2", target_bir_lowering=False)

    x_d = nc.dram_tensor("x16", [NF, S, P], f16, kind="ExternalInput")
    wq_d = nc.dram_tensor("wq", [D, H], f32, kind="ExternalInput")  # pre-scaled
    wk_d = nc.dram_tensor("wk", [D, H], f32, kind="ExternalInput")
    wv_d = nc.dram_tensor("wv", [D, H], f32, kind="ExternalInput")
    out_d = nc.dram_tensor("out", [S, H], f32, kind="ExternalOutput")

    with tile.TileContext(nc) as tc:
        with (
            tc.tile_pool(name="persist", bufs=1) as persist,
            tc.tile_pool(name="xts", bufs=2) as xts,
            tc.tile_pool(name="ptp", bufs=4) as ptp,
            tc.tile_pool(name="drainp", bufs=2) as drainp,
            tc.tile_pool(name="stp", bufs=2, space="PSUM") as stp,
            tc.tile_pool(name="op", bufs=2, space="PSUM") as opp,
            tc.tile_pool(name="scr", bufs=2, space="PSUM") as scr,
        ):
            # [q (0:64) ; k (64:128)] on partitions, s on free dim
            qkT = persist.tile([P, S], f16)
            qkTs = persist.tile([P, S], f16)      # halves swapped: [k ; q]
            v_aug = persist.tile([P, KC, VA], f16)  # [kpart, chunk, v|1|0pad]
            w_stage = persist.tile([P, 3, NF, H], f32)
            wqk_sb = persist.tile([P, NF, P], f16)  # [ Wq | Wk ] per chunk
            wv_sb = persist.tile([P, NF, H], f16)
            ident = persist.tile([P, P], f32)
            exp_bias = persist.tile([P, 1], f32)
            c3t = persist.tile([P, 1], f32)
            heat = persist.tile([P, P], f16)

            make_identity(nc, ident)
            nc.vector.memset(v_aug[:, :, H:VA], 0.0)
            nc.vector.memset(v_aug[:, :, H:H + 1], 1.0)
            nc.vector.memset(exp_bias, EXP_SHIFT)
            nc.vector.memset(c3t, E2_C3)
            if heater:
                nc.vector.memset(heat, 0.001)
            for i, w_d in enumerate((wq_d, wk_d, wv_d)):
                nc.sync.dma_start(
                    w_stage[:, i], w_d[:].rearrange("(g p) h -> p g h", p=P)
                )
            nc.vector.tensor_copy(wqk_sb[:, :, 0:H], w_stage[:, 0])
            nc.vector.tensor_copy(wqk_sb[:, :, H:P], w_stage[:, 1])
            nc.vector.tensor_copy(wv_sb[:], w_stage[:, 2])

            # ---------------- phase-1 pieces ----------------
            xf_tiles = {}

            def p1_dma(c):
                sl = slice(c * SC, (c + 1) * SC)
                xf = xts.tile([P, NF, SC], f16, tag="xf", name="xf")
                for g in range(NF):
                    nc.sync.dma_start_transpose(xf[:, g], x_d[g, sl, :])
                xf_tiles[c] = xf

            def p1_qk_half(c, half):
                xf = xf_tiles[c]
                hs = slice(half * 512, (half + 1) * 512)
                col0 = c * SC + half * 512
                ps = scr.tile([P, 512], f32, tag="scratch", name="ps")
                for g in range(NF):
                    nc.tensor.matmul(
                        ps, wqk_sb[:, g], xf[:, g, hs],
                        start=(g == 0), stop=(g == NF - 1),
                    )
                cols = slice(col0, col0 + 512)
                nc.vector.tensor_copy(qkT[:, cols], ps)
                nc.vector.tensor_copy(qkTs[0:H, cols], ps[H:P])
                nc.vector.tensor_copy(qkTs[H:P, cols], ps[0:H])

            def p1_v_piece(c, piece):
                xf = xf_tiles[c]
                for t in range(piece * (KPC // 2), (piece + 1) * (KPC // 2)):
                    ps = scr.tile([P, 512], f32, tag="scratch", name="psv")
                    for g in range(NF):
                        nc.tensor.matmul(
                            ps[:, 0:H], xf[:, g, t * P:(t + 1) * P],
                            wv_sb[:, g],
                            start=(g == 0), stop=(g == NF - 1),
                        )
                    nc.vector.tensor_copy(
                        v_aug[:, c * KPC + t, 0:H], ps[:, 0:H]
                    )
                if piece == 1:
                    xf_tiles.pop(c)

            def p1_proj_pieces(c):
                return [lambda h=h: p1_qk_half(c, h) for h in range(2)] + \
                       [lambda p=p: p1_v_piece(c, p) for p in range(2)]

            # ---------------- phase-2 emitters ----------------
            o_tiles = {}

            def emit_qk(qc, k):
                st = stp.tile([P, WIN, QC], f32, tag="st", name="st")
                if heater:
                    nc.tensor.matmul(st[:, 0, 0:P], heat, heat,
                                     start=True, stop=True)
                for j in range(WIN):
                    kj = k + j
                    hp = (kj % 2) * H
                    # k rows live at partitions 64:128 of qkT, 0:64 of qkTs;
                    # q rows at 0:64 of qkT, 64:128 of qkTs.
                    kt = qkTs if hp == 0 else qkT
                    qt = qkT if hp == 0 else qkTs
                    nc.tensor.matmul(
                        st[:, j],
                        kt[hp:hp + H, kj * P:(kj + 1) * P],
                        qt[hp:hp + H, qc * QC:(qc + 1) * QC],
                        start=True, stop=True,
                        tile_position=(hp, 0),
                    )
                return st

            def emit_exp(st, use_dve):
                pt = ptp.tile([P, WIN, QC], bf16, tag="pt", name="pt")
                if use_dve:
                    nc.vector._custom_dve(
                        EXP2, out=pt.bitcast(u16), in0=st, in1=c3t,
                        s0=E2_C0, s1=E2_C1, imm2=E2_C2,
                    )
                else:
                    nc.scalar.activation(
                        pt, st, mybir.ActivationFunctionType.Exp,
                        bias=exp_bias, scale=float(np.log(2.0) / 128.0),
                    )
                return pt

            def emit_pv(qc, k, pt):
                if k == 0:
                    o_tiles[qc] = opp.tile([P, QC], f32, tag="o", name="o_ps")
                for j in range(WIN):
                    nc.tensor.matmul(
                        o_tiles[qc], v_aug[:, k + j], pt[:, j],
                        start=(k + j == 0), stop=(k + j == KC - 1),
                        skip_group_check=True,
                    )

            def emit_drain(qc):
                o_ps = o_tiles.pop(qc)
                oT = drainp.tile([H + 1, QC], f32, tag="oT", name="oT")
                nc.vector.tensor_copy(oT, o_ps[0:H + 1])
                t_ps = scr.tile([P, 512], f32, tag="scratch", name="t_ps")
                tps = t_ps[:, 0:(QC // P) * (H + 1)].rearrange(
                    "p (j h) -> p j h", h=H + 1
                )
                if heater:
                    nc.tensor.matmul(t_ps[:, 0:P], heat, heat,
                                     start=True, stop=True)
                stage = drainp.tile([P, QC // P, H], f32, tag="stage",
                                    name="stage")
                rz = drainp.tile([P, QC // P, 1], f32, tag="rz", name="rz")
                for j in range(QC // P):
                    nc.tensor.transpose(
                        tps[:, j], oT[:, j * P:(j + 1) * P],
                        ident[:H + 1, :H + 1],
                    )
                nc.vector.reciprocal(rz, tps[:, :, H:H + 1])
                for j in range(QC // P):
                    nc.vector.tensor_scalar_mul(
                        stage[:, j], tps[:, j, 0:H], rz[:, j]
                    )
                nc.sync.dma_start(
                    out_d[qc * QC:(qc + 1) * QC, :].rearrange(
                        "(j p) h -> p j h", p=P
                    ),
                    stage,
                )

            # ---------------- schedule ----------------
            # window list: q-chunk pairs; within a pair alternate qc per k-win
            windows = []
            pair0 = (0, 1)
            for kw in range(KC // WIN):
                for qc in pair0:
                    windows.append((qc, kw * WIN))
            for pr in range(1, NQC // 2):
                pair = (2 * pr, 2 * pr + 1)
                for kw in range(KC // WIN):
                    for qc in pair:
                        windows.append((qc, kw * WIN))

            # p1 piece placement: era c = windows of pair0 covering k-chunks
            # [c*KPC, (c+1)*KPC) = window indices [c*KPC, (c+1)*KPC) (WIN=2,
            # 2 qc per kw -> KPC windows per era).
            pre_actions = {}   # idx -> list of thunks run before emit_qk
            post_actions = {}  # idx -> list run after emit_qk
            for c in range(1, NSC):
                era0 = (c - 1) * KPC
                pre_actions.setdefault(era0, []).append(lambda c=c: p1_dma(c))
                pieces = p1_proj_pieces(c)
                npieces = len(pieces)
                for pi, piece in enumerate(pieces):
                    idx = era0 + KPC - npieces + pi
                    post_actions.setdefault(idx, []).append(piece)

            with nc.named_scope("p1_c0"):
                p1_dma(0)
                for piece in p1_proj_pieces(0):
                    piece()

            prev = None
            ndve = 0
            for i, (qc, k) in enumerate(windows):
                with nc.named_scope(f"w{i}_q{qc}_k{k}"):
                    for act in pre_actions.get(i, ()):
                        act()
                    st = emit_qk(qc, k)
                    for act in post_actions.get(i, ()):
                        act()
                    if prev is not None:
                        emit_pv(prev[0], prev[1], prev[2])
                        if prev[1] + WIN == KC:
                            emit_drain(prev[0])
                    use_dve = ((i + 1) * dve_num // dve_den) > (i * dve_num // dve_den)
                    ndve += use_dve
                    pt = emit_exp(st, use_dve)
                    prev = (qc, k, pt)
            with nc.named_scope("p2_tail"):
                emit_pv(prev[0], prev[1], prev[2])
                emit_drain(prev[0])

    nc.compile()
    return nc


def make_host_inputs(x):
    """fp16 cast of x, feature-chunk-major so each [S, 128] slab is contiguous
    for the xbar DMA transpose. x: [..., S, D]."""
    s, d = x.shape[-2], x.shape[-1]
    lead = x.shape[:-2]
    nf = d // P
    x16 = x.astype(np.float16).reshape(*lead, s, nf, P).swapaxes(-2, -3)
    return np.ascontiguousarray(x16)


def kernel(x, W_q, W_k, W_v):
    from concourse.bass_utils import run_bass_kernel_spmd

    x = np.ascontiguousarray(np.asarray(x, dtype=np.float32))
    W_q = np.ascontiguousarray(np.asarray(W_q, dtype=np.float32) * np.float32(QBIT))
    W_k = np.ascontiguousarray(np.asarray(W_k, dtype=np.float32))
    W_v = np.ascontiguousarray(np.asarray(W_v, dtype=np.float32))

    x16 = make_host_inputs(x)

    if "nc" not in _cached:
        _cached["nc"] = build_program()
    nc = _cached["nc"]

    in_maps = [
        {"x16": x16[c], "wq": W_q, "wk": W_k, "wv": W_v}
        for c in range(B)
    ]
    res = run_bass_kernel_spmd(nc, in_maps, core_ids=list(range(B)))
    _cached["last_res"] = res
    return np.stack([r["out"] for r in res.results], axis=0)


if __name__ == "__main__":
    rng = np.random.default_rng(0)
    x = rng.standard_normal((B, S, D), dtype=np.float32)
    Wq = rng.standard_normal((D, H), dtype=np.float32) * D ** -0.5
    Wk = rng.standard_normal((D, H), dtype=np.float32) * D ** -0.5
    Wv = rng.standard_normal((D, H), dtype=np.float32) * D ** -0.5
    out = kernel(x, Wq, Wk, Wv)
    print(out.shape, out.dtype)


# revision 4
# speedup vs baseline: 1.1740x; 1.1398x over previous
"""Trainium2 Bass kernel for nn_CausalAttention (no actual causal mask, per the
reference bug): out = softmax((x@Wq)(x@Wk)^T / 64**0.05) @ (x@Wv).

Sharding: data-parallel over batch, one batch element per NeuronCore (B=8).

Key structure (v2 — dual-engine softmax + phase overlap):
 - Scores are produced in "bf16-bit units": W_q is pre-scaled on the HOST by
   128*log2(e)/SCALE, so the QK^T PSUM value s_b satisfies
   exp(s/SCALE) = 2^(s_b/128).  The exp is then computed on TWO engines
   concurrently (windows split between them):
     * ACT (scalar) engine: exact spline exp with scale=ln2/128, bias=-25.
     * DVE (vector) engine: a CUSTOM 8-stage ALU op (EXP2_BITS_ANT) that
       computes the bf16 BIT PATTERN of 2^(s_b/128 + c) directly:
       magic-number floor-split + quadratic mantissa-hump correction,
       written as uint16 and bit-viewed as bf16 (max rel err ~0.6%, on par
       with the bf16 quantization the exact path pays anyway).
   The scalar engine alone was the phase-2 bottleneck (100% busy at 205us).
 - 2-deep software pipeline: the PE FIFO runs QK(i+2) before PV(i), so
   exp(i) and exp(i+1) run concurrently on the two engines while the PE
   streams the previous window's PV.
 - Phase 1 (x^T DMA transposes + q/k/v projections) is interleaved into the
   first q-chunk pair's windows so the serialized xbar transposes (~1.35us
   per [1024,128] fp16 slab, one safe DMA ring) hide behind attention.
   The first s-chunk is 512 rows to shorten the startup ramp.
 - q^T/k^T are produced in ONE projection pass with stationary [Wq|Wk]
   (M=128); PSUM->SBUF copies go to the SCALAR engine, and the swapped-half
   copy needed for row-paired QK^T matmuls (two K=64 matmuls in the two PE
   row halves via tile_position) runs on GPSIMD — the DVE stays free for exp.
 - v_aug is padded to M=128 ([v | ones | zeros]) so PV matmuls are
   full-array (K=128, M=128) and keep the PE HAM clock-gate at 2.4 GHz
   without heater matmuls.
 - probabilities bf16, x fp16 (host-preformatted feature-chunk-major for the
   2-byte xbar DMA transpose), all matmuls accumulate in fp32 PSUM; softmax
   denominator comes free via the ones column (sum of the same rounded
   weights -> output stays a proper weighted average).
"""

import sys

import numpy as np

for _p in ("/root/.axon_site", "/root/.axon_site/_ro/trn_rl_repo",
           "/root/.axon_site/_ro/pypackages", "/opt/trn_rl_repo"):
    if _p not in sys.path:
        sys.path.append(_p)

B, S, D, H = 8, 4096, 768, 64
P = 128
SCALE = float(H) ** 0.05
LOG2E = 1.4426950408889634
QBIT = 128.0 * LOG2E / SCALE      # host pre-scale folded into W_q
EXP_SHIFT = -25.0                 # common shift, cancels in softmax

# EXP2_BITS_ANT constants (fp32->uint16 cast rounds to nearest, HW-verified)
E2_C2 = 0.00265                                      # mantissa-hump quad coef
E2_C0 = 128.0 * (127.0 + EXP_SHIFT * LOG2E) - 64.0   # bias - 64 (floor split)
E2_C1 = 1.5 * 2.0 ** 30                              # magic (rounds to k*128)
E2_C3 = (64.0 - 4096.0 * E2_C2) / E2_C2              # alignment const / coef

_cached = {}


def _register_exp2():
    """Register the custom DVE op computing bf16 bits of 2^((x+C0)/128)+hump.

    bits = Y2 + ((F*F) + C3)*C2 ; Y2 = x + C0 ; F = Y2 - round128(Y2)
    Exactly 8 ALU stages; C3 spilled via in1 per the custom-DVE API.
    """
    from concourse import dve_ops as dvo
    from concourse.dve_spec import (
        C0, C1, C2, C3, Spec, Src0, _spill_c3_to_src1, lower,
    )
    from concourse.dve_uop import DveOpSpec

    for op in dvo.OPS:
        if op.name == "EXP2_BITS_ANT":
            return op

    Y2 = Src0 + C0
    T = Y2 + C1
    N = T - C1
    F = Y2 - N
    Q = F * F
    bits = Y2 + (Q + C3) * C2

    def ref(in0, in1, c0, c1, c2):
        y2 = in0.astype(np.float32) + np.float32(c0)
        t = (y2 + np.float32(c1)).astype(np.float32)
        n = (t - np.float32(c1)).astype(np.float32)
        f = y2 - n
        c3 = np.asarray(in1, np.float32).reshape(in0.shape[0], 1)
        return ((f * f) + c3) * np.float32(c2) + y2

    spec = Spec(body=_spill_c3_to_src1(bits), reference=ref)
    shas = {}
    for ver in ("v3", "v4"):
        uops = lower(spec, ver=ver)
        shas[ver] = DveOpSpec(
            name="EXP2_BITS_ANT", opcode=None, uops=uops, rd1_en=True
        ).sha(ver)
    op = dvo.DveOp("EXP2_BITS_ANT", spec, subdim=False, uops_sha=shas)
    dvo.OPS.append(op)
    dvo.CUSTOM_DVE_SPECS[op.name] = op.spec
    dvo._SUB_OPCODE_FOR_NAME[op.name] = dvo._CUSTOM_DVE_ROW_BASE + len(dvo.OPS) - 1
    return op


def build_program(S=S, D=D, H=H, QC=512, WIN=2, dve_num=55, dve_den=128,
                  heater=False, qk_copy_eng="scalar", swap_eng="gpsimd"):
    import concourse.mybir as mybir
    import concourse.tile as tile
    from concourse import bacc
    from concourse.masks import make_identity

    EXP2 = _register_exp2()

    NF = D // P          # feature chunks (6)
    KC = S // P          # k-chunks (32)
    NQC = S // QC        # q-chunks (8)
    VA = P               # padded v_aug width (full-array PV)
    # phase-1 s-chunks (rows); first ones smaller for a fast startup ramp
    CH_ROWS = [512, 512, 1024, 1024, 1024]
    assert sum(CH_ROWS) == S
    CH_K = [r // P for r in CH_ROWS]          # k-chunks per p1 chunk
    CH_OFF = [sum(CH_ROWS[:i]) for i in range(len(CH_ROWS))]

    f32 = mybir.dt.float32
    f16 = mybir.dt.float16
    bf16 = mybir.dt.bfloat16
    u16 = mybir.dt.uint16

    nc = bacc.Bacc("TRN2", target_bir_lowering=False)

    x_d = nc.dram_tensor("x16", [NF, S, P], f16, kind="ExternalInput")
    wq_d = nc.dram_tensor("wq", [D, H], f32, kind="ExternalInput")  # pre-scaled
    wk_d = nc.dram_tensor("wk", [D, H], f32, kind="ExternalInput")
    wv_d = nc.dram_tensor("wv", [D, H], f32, kind="ExternalInput")
    out_d = nc.dram_tensor("out", [S, H], f32, kind="ExternalOutput")

    with tile.TileContext(nc) as tc:
        with (
            tc.tile_pool(name="persist", bufs=1) as persist,
            tc.tile_pool(name="xts", bufs=2) as xts,
            tc.tile_pool(name="ptp", bufs=4) as ptp,
            tc.tile_pool(name="drainp", bufs=2) as drainp,
            tc.tile_pool(name="stp", bufs=3, space="PSUM") as stp,
            tc.tile_pool(name="op", bufs=2, space="PSUM") as opp,
        ):
            # [q (0:64) ; k (64:128)] on partitions, s on free dim
            qkT = persist.tile([P, S], f16)
            qkTs = persist.tile([P, S], f16)      # halves swapped: [k ; q]
            v_aug = persist.tile([P, KC, VA], f16)  # [kpart, chunk, v|1|0pad]
            w_stage = persist.tile([P, 3, NF, H], f32)
            wqk_sb = persist.tile([P, NF, P], f16)  # [ Wq | Wk ] per chunk
            wv_sb = persist.tile([P, NF, H], f16)
            ident = persist.tile([P, P], f32)
            exp_bias = persist.tile([P, 1], f32)
            c3t = persist.tile([P, 1], f32)
            heat = persist.tile([P, P], f16)

            make_identity(nc, ident)
            nc.vector.memset(v_aug[:, :, H:VA], 0.0)
            nc.vector.memset(v_aug[:, :, H:H + 1], 1.0)
            nc.vector.memset(exp_bias, EXP_SHIFT)
            nc.vector.memset(c3t, E2_C3)
            if heater:
                nc.vector.memset(heat, 0.001)
            for i, w_d in enumerate((wq_d, wk_d, wv_d)):
                nc.sync.dma_start(
                    w_stage[:, i], w_d[:].rearrange("(g p) h -> p g h", p=P)
                )
            nc.vector.tensor_copy(wqk_sb[:, :, 0:H], w_stage[:, 0])
            nc.vector.tensor_copy(wqk_sb[:, :, H:P], w_stage[:, 1])
            nc.vector.tensor_copy(wv_sb[:], w_stage[:, 2])

            qk_copy = nc.scalar.copy if qk_copy_eng == "scalar" else \
                (lambda o, i_: nc.vector.tensor_copy(o, i_))
            swap_copy = nc.gpsimd.tensor_copy if swap_eng == "gpsimd" else \
                nc.vector.tensor_copy

            # ---------------- phase-1 pieces ----------------
            xf_tiles = {}

            def p1_dma(c):
                rows = CH_ROWS[c]
                sl = slice(CH_OFF[c], CH_OFF[c] + rows)
                xf = xts.tile([P, NF, 1024], f16, tag="xf", name="xf")
                for g in range(NF):
                    nc.sync.dma_start_transpose(xf[:, g, 0:rows], x_d[g, sl, :])
                xf_tiles[c] = xf

            def p1_qk_half(c, half):
                xf = xf_tiles[c]
                hs = slice(half * 512, (half + 1) * 512)
                col0 = CH_OFF[c] + half * 512
                ps = stp.tile([P, WIN, QC], f32, tag="st", name="ps")
                psf = ps.rearrange("p a b -> p (a b)")
                for g in range(NF):
                    nc.tensor.matmul(
                        psf[:, 0:512], wqk_sb[:, g], xf[:, g, hs],
                        start=(g == 0), stop=(g == NF - 1),
                    )
                cols = slice(col0, col0 + 512)
                qk_copy(qkT[:, cols], psf[:, 0:512])
                # swapped halves [k ; q] for the paired QK^T matmuls
                swap_copy(qkTs[0:H, cols], qkT[H:P, cols])
                swap_copy(qkTs[H:P, cols], qkT[0:H, cols])

            def p1_v_piece(c, piece, npieces=2):
                xf = xf_tiles[c]
                kpc = CH_K[c]
                lo = piece * kpc // npieces
                hi = (piece + 1) * kpc // npieces
                k0 = CH_OFF[c] // P
                for t in range(lo, hi):
                    ps = stp.tile([P, WIN, QC], f32, tag="st", name="psv")
                    psf = ps.rearrange("p a b -> p (a b)")
                    for g in range(NF):
                        nc.tensor.matmul(
                            psf[:, 0:H], xf[:, g, t * P:(t + 1) * P],
                            wv_sb[:, g],
                            start=(g == 0), stop=(g == NF - 1),
                        )
                    nc.vector.tensor_copy(v_aug[:, k0 + t, 0:H], psf[:, 0:H])
                if piece == npieces - 1:
                    xf_tiles.pop(c)

            def p1_proj_pieces(c):
                halves = CH_ROWS[c] // 512
                return [lambda h=h: p1_qk_half(c, h) for h in range(halves)] \
                    + [lambda p=p: p1_v_piece(c, p) for p in range(2)]

            # ---------------- phase-2 emitters ----------------
            o_tiles = {}

            def emit_qk(qc, k):
                st = stp.tile([P, WIN, QC], f32, tag="st", name="st")
                if heater:
                    nc.tensor.matmul(st[:, 0, 0:P], heat, heat,
                                     start=True, stop=True)
                for j in range(WIN):
                    kj = k + j
                    hp = (kj % 2) * H
                    # k rows: partitions 64:128 of qkT, 0:64 of qkTs;
                    # q rows: partitions 0:64 of qkT, 64:128 of qkTs.
                    kt = qkTs if hp == 0 else qkT
                    qt = qkT if hp == 0 else qkTs
                    nc.tensor.matmul(
                        st[:, j],
                        kt[hp:hp + H, kj * P:(kj + 1) * P],
                        qt[hp:hp + H, qc * QC:(qc + 1) * QC],
                        start=True, stop=True,
                        tile_position=(hp, 0),
                    )
                return st

            def emit_exp(st, use_dve):
                pt = ptp.tile([P, WIN, QC], bf16, tag="pt", name="pt")
                if use_dve:
                    nc.vector._custom_dve(
                        EXP2, out=pt.bitcast(u16), in0=st, in1=c3t,
                        s0=E2_C0, s1=E2_C1, imm2=E2_C2,
                    )
                else:
                    nc.scalar.activation(
                        pt, st, mybir.ActivationFunctionType.Exp,
                        bias=exp_bias, scale=float(np.log(2.0) / 128.0),
                    )
                return pt

            def emit_pv(qc, k, pt):
                if k == 0:
                    o_tiles[qc] = opp.tile([P, QC], f32, tag="o", name="o_ps")
                for j in range(WIN):
                    nc.tensor.matmul(
                        o_tiles[qc], v_aug[:, k + j], pt[:, j],
                        start=(k + j == 0), stop=(k + j == KC - 1),
                        skip_group_check=True,
                    )

            def emit_drain(qc):
                o_ps = o_tiles.pop(qc)
                oT = drainp.tile([H + 1, QC], f32, tag="oT", name="oT")
                nc.vector.tensor_copy(oT, o_ps[0:H + 1])
                t_ps = stp.tile([P, WIN, QC], f32, tag="st", name="t_ps")
                tps = t_ps.rearrange("p a b -> p (a b)")[
                    :, 0:(QC // P) * (H + 1)
                ].rearrange("p (j h) -> p j h", h=H + 1)
                if heater:
                    nc.tensor.matmul(
                        t_ps.rearrange("p a b -> p (a b)")[:, 0:P],
                        heat, heat, start=True, stop=True,
                    )
                stage = drainp.tile([P, QC // P, H], f32, tag="stage",
                                    name="stage")
                rz = drainp.tile([P, QC // P, 1], f32, tag="rz", name="rz")
                for j in range(QC // P):
                    nc.tensor.transpose(
                        tps[:, j], oT[:, j * P:(j + 1) * P],
                        ident[:H + 1, :H + 1],
                    )
                nc.vector.reciprocal(rz, tps[:, :, H:H + 1])
                for j in range(QC // P):
                    nc.vector.tensor_scalar_mul(
                        stage[:, j], tps[:, j, 0:H], rz[:, j]
                    )
                nc.sync.dma_start(
                    out_d[qc * QC:(qc + 1) * QC, :].rearrange(
                        "(j p) h -> p j h", p=P
                    ),
                    stage,
                )

            # ---------------- schedule ----------------
            # Window list with availability-aware ordering for the first
            # q-chunk pair (interleaved with phase-1 chunks), then plain
            # qc-alternating order for the remaining pairs.
            windows = []       # (qc, k)
            pre_actions = {}   # idx -> thunks before emit_qk
            post_actions = {}  # idx -> thunks after emit_qk

            # available k-windows per era for pair (0,1):
            # era e runs after p1 chunk e is projected; chunk e+1's DMA is
            # issued at era start, its projections at the era's tail.
            kq = {0: 0, 1: 0}        # next k (in chunks) per qc
            qava = lambda c: [qc for qc in (0, 1) if CH_OFF[c] + CH_ROWS[c] >= (qc + 1) * QC]
            for era in range(len(CH_ROWS)):
                kava = sum(CH_K[:era + 1])       # k-chunks available
                era_start = len(windows)
                newwins = []
                more = True
                while more:
                    more = False
                    for qc in qava(era):
                        if kq[qc] + WIN <= kava:
                            newwins.append((qc, kq[qc]))
                            kq[qc] += WIN
                            more = True
                windows += newwins
                if era + 1 < len(CH_ROWS):
                    pre_actions.setdefault(era_start, []).append(
                        lambda c=era + 1: p1_dma(c))
                    pieces = p1_proj_pieces(era + 1)
                    n = len(pieces)
                    for pi, piece in enumerate(pieces):
                        idx = max(era_start, len(windows) - n + pi)
                        post_actions.setdefault(idx, []).append(piece)
            assert kq == {0: KC, 1: KC}
            for pr in range(1, NQC // 2):
                pair = (2 * pr, 2 * pr + 1)
                for kw in range(KC // WIN):
                    for qc in pair:
                        windows.append((qc, kw * WIN))

            with nc.named_scope("p1_c0"):
                p1_dma(0)
                for piece in p1_proj_pieces(0):
                    piece()

            # ---- 2-deep software pipeline ----
            n = len(windows)
            use_dve = [
                ((i + 1) * dve_num // dve_den) > (i * dve_num // dve_den)
                for i in range(n)
            ]
            sts = {}
            pts = {}

            def stage_qk(i):
                qc, k = windows[i]
                with nc.named_scope(f"qk{i}_q{qc}_k{k}"):
                    for act in pre_actions.get(i, ()):
                        act()
                    sts[i] = emit_qk(qc, k)
                    for act in post_actions.get(i, ()):
                        act()

            def stage_exp(i):
                pts[i] = emit_exp(sts.pop(i), use_dve[i])

            def stage_pv(i):
                qc, k = windows[i]
                with nc.named_scope(f"pv{i}_q{qc}_k{k}"):
                    emit_pv(qc, k, pts.pop(i))
                    if k + WIN == KC:
                        emit_drain(qc)

            stage_qk(0)
            stage_qk(1)
            stage_exp(0)
            for i in range(2, n):
                stage_qk(i)
                stage_pv(i - 2)
                stage_exp(i - 1)
            with nc.named_scope("p2_tail"):
                stage_pv(n - 2)
                stage_exp(n - 1)
                stage_pv(n - 1)

    nc.compile()
    return nc


def make_host_inputs(x):
    """fp16 cast of x, feature-chunk-major so each [S, 128] slab is contiguous
    for the xbar DMA transpose. x: [..., S, D]."""
    s, d = x.shape[-2], x.shape[-1]
    lead = x.shape[:-2]
    nf = d // P
    x16 = x.astype(np.float16).reshape(*lead, s, nf, P).swapaxes(-2, -3)
    return np.ascontiguousarray(x16)


def kernel(x, W_q, W_k, W_v):
    from concourse.bass_utils import run_bass_kernel_spmd

    x = np.ascontiguousarray(np.asarray(x, dtype=np.float32))
    W_q = np.ascontiguousarray(np.asarray(W_q, dtype=np.float32) * np.float32(QBIT))
    W_k = np.ascontiguousarray(np.asarray(W_k, dtype=np.float32))
    W_v = np.ascontiguousarray(np.asarray(W_v, dtype=np.float32))

    x16 = make_host_inputs(x)

    if "nc" not in _cached:
        _cached["nc"] = build_program()
    nc = _cached["nc"]

    in_maps = [
        {"x16": x16[c], "wq": W_q, "wk": W_k, "wv": W_v}
        for c in range(B)
    ]
    res = run_bass_kernel_spmd(nc, in_maps, core_ids=list(range(B)))
    _cached["last_res"] = res
    return np.stack([r["out"] for r in res.results], axis=0)


if __name__ == "__main__":
    rng = np.random.default_rng(0)
    x = rng.standard_normal((B, S, D), dtype=np.float32)
    Wq = rng.standard_normal((D, H), dtype=np.float32) * D ** -0.5
    Wk = rng.standard_normal((D, H), dtype=np.float32) * D ** -0.5
    Wv = rng.standard_normal((D, H), dtype=np.float32) * D ** -0.5
    out = kernel(x, Wq, Wk, Wv)
    print(out.shape, out.dtype)
